# revision 2
# baseline (speedup 1.0000x reference)
"""Causal self-attention head (B=4, T=2048, C=1024, H=64) on 8 trn2 cores.

Sharding: core pair {2b, 2b+1} owns batch b. Within a pair, the sequence's
128-row chunks are zigzag-split by parity: core parity p owns chunks
{2m+p}. Each core projects K/V (and its half of Q) only for its own
chunks, the Q^T halves are AllGathered within the pair, each core runs
flash-style causal attention of ALL queries against ITS k-chunks
(producing unnormalized partial O plus softmax-denominator row via a
ones-column in V), and the partials are pair-AllReduced before the final
normalize.  This makes the instruction stream identical on all 8 cores
(pure SPMD); all per-core differences (which rows, which diagonal masks)
live in the input data.

Numerics: fp32 throughout; softmax without max-subtraction (|scores| <~ 4
for these input scales, exp is safe in fp32).
"""
import numpy as np

import concourse.bacc as bacc
import concourse.mybir as mybir
from concourse import tile, masks
from concourse.bass_utils import run_bass_kernel_spmd

F32 = mybir.dt.float32
B, T, C, H = 4, 2048, 1024, 64
KC = 128            # k-chunk rows (and T-tile rows)
TQB = 512           # query block columns
NT_OWN = T // (2 * KC)      # own 128-row tiles per core = 8
NC_CHUNKS = C // KC         # contraction chunks = 8
NBLK = T // TQB             # query blocks = 4
PAIRS = [[0, 1], [2, 3], [4, 5], [6, 7]]

_CACHE = {}


def _build_nc():
    nc = bacc.Bacc(None, num_devices=8)
    xd = nc.dram_tensor("x", [NT_OWN * KC, C], F32, kind="ExternalInput")
    wkq = nc.dram_tensor("wkq", [C, 2 * H], F32, kind="ExternalInput")
    wv = nc.dram_tensor("wv", [C, H], F32, kind="ExternalInput")
    bkq = nc.dram_tensor("bkq", [2 * H, 1], F32, kind="ExternalInput")
    bv = nc.dram_tensor("bv", [H, 1], F32, kind="ExternalInput")
    mskd = nc.dram_tensor("msk", [2, KC, TQB], F32, kind="ExternalInput")
    outd = nc.dram_tensor("out", [T, H], F32, kind="ExternalOutput")

    with tile.TileContext(nc) as tc:
        with tc.tile_pool(name="const", bufs=1) as const, \
             tc.tile_pool(name="xload", bufs=3) as xload, \
             tc.tile_pool(name="big", bufs=1) as bigp, \
             tc.tile_pool(name="ptp", bufs=3) as ptp, \
             tc.tile_pool(name="dinvp", bufs=4) as dinvp, \
             tc.tile_pool(name="psA", bufs=3, space="PSUM") as psA, \
             tc.tile_pool(name="psO", bufs=2, space="PSUM") as psO, \
             tc.tile_pool(name="psF", bufs=2, space="PSUM") as psF, \
             tc.tile_pool(name="dram", bufs=1, space="DRAM") as dram:

            ident = const.tile([128, 128], F32)
            masks.make_identity(nc, ident[:])

            wkq_sb = const.tile([KC, NC_CHUNKS, 2 * H], F32)
            nc.sync.dma_start(out=wkq_sb[:],
                              in_=wkq[:].rearrange("(n p) m -> p n m", p=KC))
            wv_sb = const.tile([KC, NC_CHUNKS, H], F32)
            nc.sync.dma_start(out=wv_sb[:],
                              in_=wv[:].rearrange("(n p) m -> p n m", p=KC))
            bkq_sb = const.tile([2 * H, 1], F32)
            nc.sync.dma_start(out=bkq_sb[:], in_=bkq[:])
            bv_sb = const.tile([H, 1], F32)
            nc.sync.dma_start(out=bv_sb[:], in_=bv[:])
            msk_sb = const.tile([KC, 2, TQB], F32)
            nc.sync.dma_start(out=msk_sb[:],
                              in_=mskd[:].rearrange("s p q -> p s q"))

            # x^T for own tiles: [c-in-chunk (partition), c-chunk, own-T]
            xT = bigp.tile([KC, NC_CHUNKS, NT_OWN * KC], F32)
            # rows 0:64 = K^T_own, rows 64:128 = Q^T_own  (both [64, 1024])
            proj = bigp.tile([128, NT_OWN * KC], F32)
            vT = bigp.tile([H, NT_OWN * KC], F32)
            vaug = bigp.tile([KC, NT_OWN, H + 1], F32)
            qTf = bigp.tile([H, T], F32)
            o_part = bigp.tile([H + 1, T], F32)
            o_full = bigp.tile([H + 1, T], F32)

            qg_in = dram.tile([H, NT_OWN * KC], F32)
            qg_out = dram.tile([2 * H, NT_OWN * KC], F32)
            ar_in = dram.tile([H + 1, T], F32)
            ar_out = dram.tile([H + 1, T], F32)

            # ---- phase A+B: load, transpose, project (two 512-col groups)
            for half in range(2):
                for t in range(half * 4, half * 4 + 4):
                    x_sb = xload.tile([KC, C], F32)
                    nc.sync.dma_start(out=x_sb[:],
                                      in_=xd[t * KC:(t + 1) * KC, :])
                    for cb in range(2):
                        pst = psA.tile([128, 512], F32, tag="A")
                        for cc in range(4):
                            nc.tensor.transpose(
                                pst[:, cc * KC:(cc + 1) * KC],
                                x_sb[:, (4 * cb + cc) * KC:(4 * cb + cc + 1) * KC],
                                ident[:])
                        nc.vector.tensor_copy(
                            out=xT[:, 4 * cb:4 * cb + 4, t * KC:(t + 1) * KC],
                            in_=pst[:].rearrange("p (n t) -> p n t", t=KC))
                col = slice(half * TQB, (half + 1) * TQB)
                ps_kq = psA.tile([128, 512], F32, tag="A")
                for c in range(NC_CHUNKS):
                    nc.tensor.matmul(ps_kq[:], wkq_sb[:, c, :], xT[:, c, col],
                                     start=(c == 0), stop=(c == NC_CHUNKS - 1))
                nc.scalar.activation(proj[:, col], ps_kq[:],
                                     mybir.ActivationFunctionType.Identity,
                                     bias=bkq_sb[:])
                ps_v = psA.tile([128, 512], F32, tag="A")
                for c in range(NC_CHUNKS):
                    nc.tensor.matmul(ps_v[0:H, :], wv_sb[:, c, :], xT[:, c, col],
                                     start=(c == 0), stop=(c == NC_CHUNKS - 1))
                nc.scalar.activation(vT[:, col], ps_v[0:H, :],
                                     mybir.ActivationFunctionType.Identity,
                                     bias=bv_sb[:])

            # ---- V^T -> V (natural layout), augmented with a ones column
            ps_vt = psA.tile([128, 512], F32, tag="A")
            for m in range(NT_OWN):
                nc.tensor.transpose(ps_vt[:, m * H:(m + 1) * H],
                                    vT[:, m * KC:(m + 1) * KC],
                                    ident[0:H, 0:H])
            nc.vector.tensor_copy(
                out=vaug[:, :, 0:H],
                in_=ps_vt[:].rearrange("p (n h) -> p n h", h=H))
            nc.gpsimd.memset(vaug[:, :, H:H + 1], 1.0)

            # ---- Q^T exchange within the pair
            nc.sync.dma_start(out=qg_in[:], in_=proj[64:128, :])
            nc.gpsimd.collective_compute(
                "AllGather", mybir.AluOpType.bypass,
                replica_groups=PAIRS, ins=[qg_in[:]], outs=[qg_out[:]])
            qTf_v = qTf[:].rearrange("p (n s t) -> p n s t", s=2, t=KC)
            qg_v = qg_out[:].rearrange("p (n t) -> p n t", t=KC)
            nc.sync.dma_start(out=qTf_v[:, :, 0], in_=qg_v[0:H])
            nc.sync.dma_start(out=qTf_v[:, :, 1], in_=qg_v[H:2 * H])

            # ---- attention: all query blocks vs own k-chunks
            for i in range(NBLK):
                q0 = i * TQB
                nchunks = 2 * i + 2
                ps_o = psO.tile([H + 1, TQB], F32)
                for m in range(nchunks):
                    ps_s = psA.tile([128, 512], F32, tag="A")
                    nc.tensor.matmul(ps_s[:], proj[0:H, m * KC:(m + 1) * KC],
                                     qTf[:, q0:q0 + TQB],
                                     start=True, stop=True)
                    ptile = ptp.tile([KC, TQB], F32)
                    nc.scalar.activation(ptile[:], ps_s[:],
                                         mybir.ActivationFunctionType.Exp,
                                         scale=float(1.0 / np.sqrt(H)))
                    if m >= 2 * i:
                        nc.gpsimd.tensor_tensor(ptile[:], ptile[:],
                                                msk_sb[:, m - 2 * i, :],
                                                op=mybir.AluOpType.mult)
                    nc.tensor.matmul(ps_o[:], vaug[:, m, :], ptile[:],
                                     start=(m == 0), stop=(m == nchunks - 1))
                nc.scalar.activation(o_part[:, q0:q0 + TQB], ps_o[:],
                                     mybir.ActivationFunctionType.Identity)

            # ---- pair AllReduce of unnormalized O (and denominators)
            nc.sync.dma_start(out=ar_in[:], in_=o_part[:])
            nc.gpsimd.collective_compute(
                "AllReduce", mybir.AluOpType.add,
                replica_groups=PAIRS, ins=[ar_in[:]], outs=[ar_out[:]])
            nc.sync.dma_start(out=o_full[:], in_=ar_out[:])

            # ---- normalize + transpose back + store
            for i in range(NBLK):
                q0 = i * TQB
                o_blk = ptp.tile([128, 4, H], F32, tag="oblk")
                for r in range(4):
                    ps_f = psF.tile([128, H + 1], F32)
                    nc.tensor.transpose(ps_f[:],
                                        o_full[:, q0 + r * 128:q0 + (r + 1) * 128],
                                        ident[0:H + 1, 0:H + 1])
                    dinv = dinvp.tile([128, 1], F32)
                    nc.vector.reciprocal(dinv[:], ps_f[:, H:H + 1])
                    nc.scalar.activation(o_blk[:, r, :], ps_f[:, 0:H],
                                         mybir.ActivationFunctionType.Identity,
                                         scale=dinv[:])
                nc.sync.dma_start(
                    out=outd[q0:q0 + TQB, :].rearrange("(r p) h -> p r h", p=128),
                    in_=o_blk[:])
    nc.finalize()
    return nc


def _get_nc():
    if "nc" not in _CACHE:
        _CACHE["nc"] = _build_nc()
    return _CACHE["nc"]


def _host_inputs(inputs, Wq, bq, Wk, bk, Wv, bv):
    x = np.ascontiguousarray(np.asarray(inputs, dtype=np.float32))
    wkq = np.ascontiguousarray(
        np.concatenate([np.asarray(Wk), np.asarray(Wq)], axis=1).astype(np.float32))
    wv = np.ascontiguousarray(np.asarray(Wv, dtype=np.float32))
    bkq = np.concatenate([np.asarray(bk), np.asarray(bq)]).astype(np.float32)[:, None]
    bvv = np.asarray(bv, dtype=np.float32)[:, None]

    in_maps = []
    for core in range(8):
        b, p = core // 2, core & 1
        xb = x[b].reshape(T // KC, KC, C)
        x_own = np.ascontiguousarray(xb[p::2].reshape(NT_OWN * KC, C))
        msk = np.zeros((2, KC, TQB), dtype=np.float32)
        for s in range(2):
            j = p + 2 * s
            q = np.arange(TQB)[None, :]
            k = np.arange(KC)[:, None]
            msk[s] = (q - k - KC * j >= 0).astype(np.float32)
        in_maps.append({"x": x_own, "wkq": wkq, "wv": wv,
                        "bkq": bkq, "bv": bvv, "msk": msk})
    return in_maps


def kernel(inputs, Wq, bq, Wk, bk, Wv, bv):
    nc = _get_nc()
    in_maps = _host_inputs(inputs, Wq, bq, Wk, bk, Wv, bv)
    res = run_bass_kernel_spmd(nc, in_maps, list(range(8))).results
    out = np.stack([res[2 * b]["out"] for b in range(B)], axis=0)
    return out.astype(np.float32)


def run_traced(inputs, Wq, bq, Wk, bk, Wv, bv, trace_cores=None):
    """Like kernel() but returns (out, BassKernelResults) with NTFF timing."""
    nc = _get_nc()
    in_maps = _host_inputs(inputs, Wq, bq, Wk, bk, Wv, bv)
    r = run_bass_kernel_spmd(nc, in_maps, list(range(8)), trace=True,
                             trace_cores=trace_cores)
    out = np.stack([r.results[2 * b]["out"] for b in range(B)], axis=0)
    return out.astype(np.float32), r


# revision 5
# speedup vs baseline: 1.2948x; 1.2948x over previous
"""Causal self-attention head (B=4, T=2048, C=1024, H=64) on 8 trn2 cores.

Sharding: core pair {2b, 2b+1} owns batch b. Within a pair, the sequence's
128-row chunks are zigzag-split by parity: core parity p owns chunks
{2m+p}. Each core projects K/V (and its half of Q) only for its own
chunks, the Q^T halves are AllGathered within the pair, each core runs
flash-style causal attention of ALL queries against ITS k-chunks
(producing unnormalized partial O plus softmax-denominator row via a
ones-column in V), and the partials are pair-AllReduced before the final
normalize.  This makes the instruction stream identical on all 8 cores
(pure SPMD); all per-core differences (which rows, which diagonal masks)
live in the input data.

Numerics: fp32 throughout; softmax without max-subtraction (|scores| <~ 4
for these input scales, exp is safe in fp32).
"""
import numpy as np

import concourse.bacc as bacc
import concourse.mybir as mybir
from concourse import tile, masks
from concourse.bass_utils import run_bass_kernel_spmd

F32 = mybir.dt.float32
F32R = mybir.dt.float32r
B, T, C, H = 4, 2048, 1024, 64
KC = 128            # k-chunk rows (and T-tile rows)
TQB = 512           # query block columns
NT_OWN = T // (2 * KC)      # own 128-row tiles per core = 8
NC_CHUNKS = C // KC         # contraction chunks = 8
NBLK = T // TQB             # query blocks = 4
PAIRS = [[0, 1], [2, 3], [4, 5], [6, 7]]

_CACHE = {}


def _build_nc():
    nc = bacc.Bacc(None, num_devices=8)
    xd = nc.dram_tensor("x", [NT_OWN * KC, C], F32, kind="ExternalInput")
    wkq = nc.dram_tensor("wkq", [C, 2 * H], F32, kind="ExternalInput")
    wv = nc.dram_tensor("wv", [C, H], F32, kind="ExternalInput")
    bkq = nc.dram_tensor("bkq", [2 * H, 1], F32, kind="ExternalInput")
    bv = nc.dram_tensor("bv", [H, 1], F32, kind="ExternalInput")
    mskd = nc.dram_tensor("msk", [2, KC, TQB], F32, kind="ExternalInput")
    outd = nc.dram_tensor("out", [T, H], F32, kind="ExternalOutput")

    with tile.TileContext(nc) as tc:
        with tc.tile_pool(name="const", bufs=1) as const, \
             tc.tile_pool(name="xload", bufs=3) as xload, \
             tc.tile_pool(name="big", bufs=1) as bigp, \
             tc.tile_pool(name="ptp", bufs=3) as ptp, \
             tc.tile_pool(name="dinvp", bufs=4) as dinvp, \
             tc.tile_pool(name="psA", bufs=3, space="PSUM") as psA, \
             tc.tile_pool(name="psO", bufs=2, space="PSUM") as psO, \
             tc.tile_pool(name="psF", bufs=2, space="PSUM") as psF, \
             tc.tile_pool(name="dram", bufs=1, space="DRAM") as dram:

            ident = const.tile([128, 128], F32)
            masks.make_identity(nc, ident[:])

            wkq_sb = const.tile([KC, NC_CHUNKS, 2 * H], F32R)
            nc.sync.dma_start(out=wkq_sb[:],
                              in_=wkq[:].rearrange("(n p) m -> p n m", p=KC).bitcast(F32R))
            wv_sb = const.tile([KC, NC_CHUNKS, H], F32R)
            nc.sync.dma_start(out=wv_sb[:],
                              in_=wv[:].rearrange("(n p) m -> p n m", p=KC).bitcast(F32R))
            bkq_sb = const.tile([2 * H, 1], F32)
            nc.sync.dma_start(out=bkq_sb[:], in_=bkq[:])
            bv_sb = const.tile([H, 1], F32)
            nc.sync.dma_start(out=bv_sb[:], in_=bv[:])
            msk_sb = const.tile([KC, 2, TQB], F32R)
            nc.sync.dma_start(out=msk_sb[:],
                              in_=mskd[:].rearrange("s p q -> p s q").bitcast(F32R))

            # x^T for own tiles: [c-in-chunk (partition), c-chunk, own-T]
            xT = bigp.tile([KC, NC_CHUNKS, NT_OWN * KC], F32R)
            # rows 0:64 = K^T_own, rows 64:128 = Q^T_own  (both [64, 1024])
            proj = bigp.tile([128, NT_OWN * KC], F32R)
            vT = bigp.tile([H, NT_OWN * KC], F32R)
            vaug = bigp.tile([KC, NT_OWN, H + 1], F32R)
            qTf = bigp.tile([H, T], F32R)
            o_part = bigp.tile([H + 1, T], F32)
            o_full = bigp.tile([H + 1, T], F32)

            qg_in = dram.tile([H, NT_OWN * KC], F32)
            qg_out = dram.tile([2 * H, NT_OWN * KC], F32)
            ar_in = dram.tile([H + 1, T], F32)
            ar_out = dram.tile([H + 1, T], F32)

            # ---- phase A+B: load, transpose, project (two 512-col groups)
            for half in range(2):
                for t in range(half * 4, half * 4 + 4):
                    x_sb = xload.tile([KC, C], F32)
                    nc.sync.dma_start(out=x_sb[:],
                                      in_=xd[t * KC:(t + 1) * KC, :])
                    for cb in range(2):
                        pst = psA.tile([128, 512], F32, tag="A")
                        for cc in range(4):
                            nc.tensor.transpose(
                                pst[:, cc * KC:(cc + 1) * KC],
                                x_sb[:, (4 * cb + cc) * KC:(4 * cb + cc + 1) * KC],
                                ident[:])
                        nc.vector.tensor_copy(
                            out=xT[:, 4 * cb:4 * cb + 4, t * KC:(t + 1) * KC],
                            in_=pst[:].rearrange("p (n t) -> p n t", t=KC))
                col = slice(half * TQB, (half + 1) * TQB)
                ps_kq = psA.tile([128, 512], F32, tag="A")
                for c in range(NC_CHUNKS):
                    nc.tensor.matmul(ps_kq[:], wkq_sb[:, c, :],
                                     xT[:, c, col],
                                     start=(c == 0), stop=(c == NC_CHUNKS - 1))
                nc.scalar.activation(proj[:, col], ps_kq[:],
                                     mybir.ActivationFunctionType.Identity,
                                     bias=bkq_sb[:])
                ps_v = psA.tile([128, 512], F32, tag="A")
                for c in range(NC_CHUNKS):
                    nc.tensor.matmul(ps_v[0:H, :], wv_sb[:, c, :],
                                     xT[:, c, col],
                                     start=(c == 0), stop=(c == NC_CHUNKS - 1))
                nc.scalar.activation(vT[:, col], ps_v[0:H, :],
                                     mybir.ActivationFunctionType.Identity,
                                     bias=bv_sb[:])

            # ---- V^T -> V (natural layout), augmented with a ones column
            ps_vt = psA.tile([128, 512], F32, tag="A")
            for m in range(NT_OWN):
                nc.tensor.transpose(ps_vt[:, m * H:(m + 1) * H],
                                    vT[:, m * KC:(m + 1) * KC].bitcast(F32),
                                    ident[0:H, 0:H])
            nc.vector.tensor_copy(
                out=vaug[:, :, 0:H],
                in_=ps_vt[:].rearrange("p (n h) -> p n h", h=H))
            nc.gpsimd.memset(vaug[:, :, H:H + 1].bitcast(F32), 1.0)

            # ---- Q^T exchange within the pair
            nc.sync.dma_start(out=qg_in[:], in_=proj[64:128, :].bitcast(F32))
            nc.gpsimd.collective_compute(
                "AllGather", mybir.AluOpType.bypass,
                replica_groups=PAIRS, ins=[qg_in[:]], outs=[qg_out[:]])
            qTf_v = qTf[:].rearrange("p (n s t) -> p n s t", s=2, t=KC)
            qg_v = qg_out[:].rearrange("p (n t) -> p n t", t=KC)
            nc.sync.dma_start(out=qTf_v[:, :, 0], in_=qg_v[0:H].bitcast(F32R))
            nc.sync.dma_start(out=qTf_v[:, :, 1], in_=qg_v[H:2 * H].bitcast(F32R))

            # ---- attention: all query blocks vs own k-chunks
            for i in range(NBLK):
                q0 = i * TQB
                nchunks = 2 * i + 2
                ps_o = psO.tile([H + 1, TQB], F32)
                for m in range(nchunks):
                    ps_s = psA.tile([128, 512], F32, tag="A")
                    nc.tensor.matmul(ps_s[:],
                                     proj[0:H, m * KC:(m + 1) * KC],
                                     qTf[:, q0:q0 + TQB],
                                     start=True, stop=True)
                    ptile = ptp.tile([KC, TQB], F32R)
                    nc.scalar.activation(ptile[:], ps_s[:],
                                         mybir.ActivationFunctionType.Exp,
                                         scale=float(1.0 / np.sqrt(H)))
                    if m >= 2 * i:
                        nc.gpsimd.tensor_tensor(ptile[:], ptile[:],
                                                msk_sb[:, m - 2 * i, :],
                                                op=mybir.AluOpType.mult)
                    nc.tensor.matmul(ps_o[:], vaug[:, m, :],
                                     ptile[:],
                                     start=(m == 0), stop=(m == nchunks - 1))
                nc.scalar.activation(o_part[:, q0:q0 + TQB], ps_o[:],
                                     mybir.ActivationFunctionType.Identity)

            # ---- pair AllReduce of unnormalized O (and denominators)
            nc.sync.dma_start(out=ar_in[:], in_=o_part[:])
            nc.gpsimd.collective_compute(
                "AllReduce", mybir.AluOpType.add,
                replica_groups=PAIRS, ins=[ar_in[:]], outs=[ar_out[:]])
            nc.sync.dma_start(out=o_full[:], in_=ar_out[:])

            # ---- normalize + transpose back + store
            for i in range(NBLK):
                q0 = i * TQB
                o_blk = ptp.tile([128, 4, H], F32, tag="oblk")
                for r in range(4):
                    ps_f = psF.tile([128, H + 1], F32)
                    nc.tensor.transpose(ps_f[:],
                                        o_full[:, q0 + r * 128:q0 + (r + 1) * 128],
                                        ident[0:H + 1, 0:H + 1])
                    dinv = dinvp.tile([128, 1], F32)
                    nc.vector.reciprocal(dinv[:], ps_f[:, H:H + 1])
                    nc.scalar.activation(o_blk[:, r, :], ps_f[:, 0:H],
                                         mybir.ActivationFunctionType.Identity,
                                         scale=dinv[:])
                nc.sync.dma_start(
                    out=outd[q0:q0 + TQB, :].rearrange("(r p) h -> p r h", p=128),
                    in_=o_blk[:])
    nc.finalize()
    return nc


def _get_nc():
    if "nc" not in _CACHE:
        _CACHE["nc"] = _build_nc()
    return _CACHE["nc"]


def _host_inputs(inputs, Wq, bq, Wk, bk, Wv, bv):
    x = np.ascontiguousarray(np.asarray(inputs, dtype=np.float32))
    wkq = np.ascontiguousarray(
        np.concatenate([np.asarray(Wk), np.asarray(Wq)], axis=1).astype(np.float32))
    wv = np.ascontiguousarray(np.asarray(Wv, dtype=np.float32))
    bkq = np.concatenate([np.asarray(bk), np.asarray(bq)]).astype(np.float32)[:, None]
    bvv = np.asarray(bv, dtype=np.float32)[:, None]

    in_maps = []
    for core in range(8):
        b, p = core // 2, core & 1
        xb = x[b].reshape(T // KC, KC, C)
        x_own = np.ascontiguousarray(xb[p::2].reshape(NT_OWN * KC, C))
        msk = np.zeros((2, KC, TQB), dtype=np.float32)
        for s in range(2):
            j = p + 2 * s
            q = np.arange(TQB)[None, :]
            k = np.arange(KC)[:, None]
            msk[s] = (q - k - KC * j >= 0).astype(np.float32)
        in_maps.append({"x": x_own, "wkq": wkq, "wv": wv,
                        "bkq": bkq, "bv": bvv, "msk": msk})
    return in_maps


def kernel(inputs, Wq, bq, Wk, bk, Wv, bv):
    nc = _get_nc()
    in_maps = _host_inputs(inputs, Wq, bq, Wk, bk, Wv, bv)
    res = run_bass_kernel_spmd(nc, in_maps, list(range(8))).results
    out = np.stack([res[2 * b]["out"] for b in range(B)], axis=0)
    return out.astype(np.float32)


def run_traced(inputs, Wq, bq, Wk, bk, Wv, bv, trace_cores=None):
    """Like kernel() but returns (out, BassKernelResults) with NTFF timing."""
    nc = _get_nc()
    in_maps = _host_inputs(inputs, Wq, bq, Wk, bk, Wv, bv)
    r = run_bass_kernel_spmd(nc, in_maps, list(range(8)), trace=True,
                             trace_cores=trace_cores)
    out = np.stack([r.results[2 * b]["out"] for b in range(B)], axis=0)
    return out.astype(np.float32), r
